# revision 36
# baseline (speedup 1.0000x reference)
"""Trainium2 Bass kernel for the 3-layer AR GRU (nn_AR_RNN_GRU).

Strategy
--------
The time recurrence is strictly sequential (127 dependent steps x 3 layers),
and cross-core exchange on this part costs more than it saves (ncfw collective
floor ~5-9us vs ~3us of per-layer compute; remote SBUF-to-SBUF DMA is not
available under this runtime).  So the whole recurrence runs on ONE core with
the full batch of 64, organized to keep the PE and the vector engines busy:

 * "Folded" layout: a [64, 768] activation lives as [128, 384] in SBUF --
   batch on partitions 0-63 for units 0-383 and partitions 64-127 for units
   384-767.  All elementwise gate math then uses the full 128 lanes, and each
   weight matrix streams as two 1152-column halves through the two PE
   column-group pairs concurrently (tile_position via psum base partition).
 * Weights and matmul stationaries live in SBUF as fp16 (fp32 does not fit
   in SBUF; fp16 keeps the 127-step compounded error ~1e-2 where bf16 gave
   ~8e-2); the folded recurrent state and all PSUM accumulation stay fp32.
 * Gate pre-activations: psum_zr accumulates x@Wx + h@Wh for the z,r gates
   (the add is free in PSUM); the candidate keeps xh and hh separate so that
   hc = tanh(xh + r*hh) matches the reset_after GRU cell.
 * The AR feedback (normalize + dense) is folded into one effective matrix:
   gx0 = p2 @ (Wd @ (Wx0/std)) + beff, which removes the dense+normalize from
   the critical path; the actual prediction p2 @ Wd + bd is computed off-path.
 * h is re-transposed each layer via 3 PE-transposes of [128,128] tiles; the
   transposed tiles double as the lhsT (stationary) for the next matmuls and
   for the dense readout.
Biases are applied exactly via an extra "ones" K-chunk whose rhs row 0 holds
the bias vector -- emitted only when the bias is nonzero (in this problem all
bi/br/bd are zero; beff is nonzero and always emitted).
"""

import os
import sys

import numpy as np

try:
    import concourse.bass as bass  # noqa: F401
except ImportError:  # grading env fallback
    sys.path.insert(0, "/opt/trn_rl_repo")

import ml_dtypes

import concourse.bass as bass
import concourse.mybir as mybir
import concourse.tile as tile
from concourse import bacc
from concourse.bass_utils import run_bass_kernel_spmd
from concourse.masks import make_identity

BF16 = np.float16

B = 64  # batch
D = 512  # data dim
U = 768  # GRU units
G = 3 * U  # gate columns
HALF = U // 2  # 384

T_IN = int(os.environ.get("GRU_TIN", "64"))
T_OUT = int(os.environ.get("GRU_TOUT", "64"))

# column permutation: [z_lo r_lo h_lo | z_hi r_hi h_hi], each block 384 wide
_PERM = np.concatenate(
    [
        np.arange(0, HALF),
        np.arange(U, U + HALF),
        np.arange(2 * U, 2 * U + HALF),
        np.arange(HALF, U),
        np.arange(U + HALF, 2 * U),
        np.arange(2 * U + HALF, G),
    ]
)


def _prep_weight(w, bias):
    """[K, 2304] fp32 (+bias [2304]) -> ([n_chunks, 128, 2304] bf16, has_bias)."""
    k = w.shape[0]
    assert k % 128 == 0
    wp = np.ascontiguousarray(w[:, _PERM]).reshape(k // 128, 128, G)
    has_bias = bias is not None and float(np.abs(bias).max()) > 0.0
    if has_bias:
        bc = np.zeros((1, 128, G), np.float32)
        bc[0, 0, :] = bias[_PERM]
        wp = np.concatenate([wp, bc], axis=0)
    return wp.astype(BF16), has_bias


def _fold(a):
    """[64, 768] -> folded [128, 384]."""
    return np.concatenate([a[:, :HALF], a[:, HALF:]], axis=0)


def _build(n_warm, n_ar, bias_flags):
    """Build the Bass program. bias_flags: dict name->bool for extra chunks."""
    nc = bacc.Bacc(num_devices=1, name="gru_ar")
    f32, bf16 = mybir.dt.float32, mybir.dt.float16
    n_steps = n_warm + n_ar

    # ---- DRAM I/O ----
    def wchunks(name, kc):
        return nc.dram_tensor(name, [kc * 128, G], bf16, kind="ExternalInput"), kc

    wx0, wx0_c = wchunks("wx0", 4 + bias_flags["bi0"])
    weff, weff_c = wchunks("weff", 6 + bias_flags["beff"])
    wx = [None, *(wchunks(f"wx{j}", 6 + bias_flags[f"bi{j}"]) for j in (1, 2))]
    wh = [wchunks(f"wh{j}", 6 + bias_flags[f"br{j}"]) for j in (0, 1, 2)]
    wd_c = 6 + bias_flags["bd"]
    wd = nc.dram_tensor("wd", [wd_c * 128, D], bf16, kind="ExternalInput")
    xt = nc.dram_tensor("xt", [n_warm * 4 * 128, B], bf16, kind="ExternalInput")
    h0f = nc.dram_tensor("h0f", [3 * 128, HALF], f32, kind="ExternalInput")
    h0t = nc.dram_tensor("h0t", [3 * 128, 3 * 128], bf16, kind="ExternalInput")
    ones = nc.dram_tensor("ones", [128, 128], bf16, kind="ExternalInput")
    out = nc.dram_tensor("out", [B, n_ar + 1, D], f32, kind="ExternalOutput")
    debug = os.environ.get("GRU_DEBUG", "") == "1"
    if debug:
        dbg_zr = nc.dram_tensor("dbg_zr", [128, 2 * HALF], f32, kind="ExternalOutput")
        dbg_hf = nc.dram_tensor("dbg_hf", [128, HALF], f32, kind="ExternalOutput")
        dbg_xt = nc.dram_tensor("dbg_xt", [128, 4 * B], f32, kind="ExternalOutput")
        dbg_ht = nc.dram_tensor("dbg_ht", [128, 3 * 128], f32, kind="ExternalOutput")

    with tile.TileContext(nc) as tc:
        with (
            tc.tile_pool(name="wpool", bufs=1) as wpool,
            tc.tile_pool(name="state", bufs=1) as spool,
            tc.tile_pool(name="work", bufs=2) as work,
            tc.tile_pool(name="workb", bufs=2) as workb,
            tc.tile_pool(name="xs", bufs=3) as xs,
            tc.tile_pool(name="pzr", bufs=2, space="PSUM") as pzr,
            tc.tile_pool(name="ph", bufs=2, space="PSUM") as ph,
            tc.tile_pool(name="paux", bufs=2, space="PSUM") as paux,
        ):
            # ---- load weights / constants ----
            def load_w(dram, kc, ncols):
                t = wpool.tile([128, kc * ncols], bf16, tag=dram.name)
                for c in range(kc):
                    nc.sync.dma_start(
                        t[:, c * ncols : (c + 1) * ncols],
                        dram[c * 128 : (c + 1) * 128, :],
                    )
                return t

            # Wx0 (warmup) and Weff (AR) share one SBUF slot; Weff is DMA'd
            # over Wx0 after the last warmup gx0 matmul (Tile orders the WAR).
            # small step-0 state/constants first, then weights in the order
            # the first step consumes them (wx0, wh0, wx1, wh1, ...), so the
            # PE can start while the rest of the ~21 MB is still in flight
            ones_t = wpool.tile([128, 128], bf16, tag="ones")
            nc.sync.dma_start(ones_t[:], ones[:])
            ident = wpool.tile([128, 128], f32, tag="ident")
            make_identity(nc, ident[:])
            hF = []
            hT = []
            for j in range(3):
                f = spool.tile([128, HALF], f32, tag=f"hF{j}")
                nc.sync.dma_start(f[:], h0f[j * 128 : (j + 1) * 128, :])
                hF.append(f)
                t = spool.tile([128, 3 * 128], bf16, tag=f"hT{j}")
                nc.sync.dma_start(t[:], h0t[j * 128 : (j + 1) * 128, :])
                hT.append(t)

            nshare = max(wx0_c, weff_c)
            wshare_t = wpool.tile([128, nshare * G], bf16, tag="wx0weff")
            for c in range(wx0_c):
                nc.sync.dma_start(
                    wshare_t[:, c * G : (c + 1) * G], wx0[c * 128 : (c + 1) * 128, :]
                )
            wx0_t = wshare_t
            weff_t = wshare_t
            wh_t = [None, None, None]
            wx_t = [None, None, None]
            wh_t[0] = load_w(wh[0][0], wh[0][1], G)
            # prefetch the first two warmup x tiles ahead of the bulk weights
            xpre = {}
            for tt in range(min(2, n_warm)):
                xtile = xs.tile([128, 4 * B], bf16, tag="xt")
                for c in range(4):
                    nc.sync.dma_start(
                        xtile[:, c * B : (c + 1) * B],
                        xt[tt * 512 + c * 128 : tt * 512 + (c + 1) * 128, :],
                    )
                xpre[tt] = xtile
            wx_t[1] = load_w(wx[1][0], wx[1][1], G)
            wh_t[1] = load_w(wh[1][0], wh[1][1], G)
            wx_t[2] = load_w(wx[2][0], wx[2][1], G)
            wh_t[2] = load_w(wh[2][0], wh[2][1], G)
            wd_t = load_w(wd, wd_c, D)

            def ht_slice(t, c):
                # K-chunk c (0..5) of the transposed folded state tile set
                if c < 3:
                    return t[:, c * 128 : c * 128 + 64]
                return t[:, (c - 3) * 128 + 64 : (c - 3) * 128 + 128]

            # ---- one recurrent step ----
            def stream(psum_zr, psum_h, w_t, kc, lhsT_fn, first_zr, last_zr):
                """Emit the matmuls of one weight stream (both folded halves).
                6-chunk streams go in order 0,3,1,4,2,5 so that each transposed
                state tile T_c unlocks its two K-chunks (c, c+3) as it lands."""
                order = [0, 3, 1, 4, 2, 5] + list(range(6, kc)) if kc >= 6 else list(range(kc))
                for ci, c in enumerate(order):
                    lhsT = lhsT_fn(c)
                    for h_ix in range(2):
                        base = 64 * h_ix
                        off = h_ix * (G // 2)
                        first = first_zr and ci == 0
                        last = last_zr and ci == kc - 1
                        nc.tensor.matmul(
                            psum_zr[base : base + 64, 0:512],
                            lhsT,
                            w_t[:, c * G + off : c * G + off + 512],
                            start=first,
                            stop=last,
                        )
                        nc.tensor.matmul(
                            psum_zr[base : base + 64, 512:768],
                            lhsT,
                            w_t[:, c * G + off + 512 : c * G + off + 768],
                            start=first,
                            stop=last,
                        )
                        nc.tensor.matmul(
                            psum_h[base : base + 64, 0:HALF],
                            lhsT,
                            w_t[:, c * G + off + 768 : c * G + off + 1152],
                            start=ci == 0,
                            stop=ci == kc - 1,
                        )

            def gru_layer(j, gx_w, gx_kc, gx_lhsT_fn):
                psum_zr = pzr.tile([128, 2 * HALF], f32, tag="zr")
                psum_xh = ph.tile([128, HALF], f32, tag="xh")
                psum_hh = paux.tile([128, HALF], f32, tag="aux")
                # recurrent stream first (inputs available earliest)
                wh_kc = wh[j][1]

                def gh_lhsT(c):
                    return ones_t[:, 0:64] if c >= 6 else ht_slice(hT[j], c)

                stream(psum_zr, psum_hh, wh_t[j], wh_kc, gh_lhsT, True, False)
                stream(psum_zr, psum_xh, gx_w, gx_kc, gx_lhsT_fn, False, True)

                # gates (folded [128, *])
                zr = work.tile([128, 2 * HALF], f32, tag="zr_s")
                # r first: it gates the critical path (r*hh); z can lag
                nc.scalar.activation(
                    zr[:, HALF : 2 * HALF],
                    psum_zr[:, HALF : 2 * HALF],
                    mybir.ActivationFunctionType.Sigmoid,
                )
                t1 = workb.tile([128, HALF], f32, tag="t1")
                nc.vector.tensor_mul(t1[:], zr[:, HALF : 2 * HALF], psum_hh[:])
                nc.scalar.activation(
                    zr[:, 0:HALF],
                    psum_zr[:, 0:HALF],
                    mybir.ActivationFunctionType.Sigmoid,
                )
                if debug and j == 0:
                    nc.sync.dma_start(dbg_zr[:], zr[:])
                nc.vector.tensor_add(t1[:], t1[:], psum_xh[:])
                hc = workb.tile([128, HALF], f32, tag="hc")
                nc.scalar.activation(hc[:], t1[:], mybir.ActivationFunctionType.Tanh)
                # h_new = hc + z*(h_prev - hc)
                d = workb.tile([128, HALF], f32, tag="d")
                nc.vector.tensor_sub(d[:], hF[j][:], hc[:])
                nc.vector.tensor_mul(d[:], zr[:, 0:HALF], d[:])
                nc.vector.tensor_add(hF[j][:], d[:], hc[:])
                # re-transpose the folded state for the next matmuls
                for c in range(3):
                    ptr = ph.tile([128, 128], f32, tag="xh")
                    nc.tensor.transpose(
                        ptr[:], hF[j][:, c * 128 : (c + 1) * 128], ident[:]
                    )
                    # alternate engines so psum->sbuf copies run in parallel
                    if c == 1:
                        nc.vector.tensor_copy(hT[j][:, c * 128 : (c + 1) * 128], ptr[:])
                    else:
                        nc.scalar.copy(hT[j][:, c * 128 : (c + 1) * 128], ptr[:])
                if debug and j == 0:
                    nc.sync.dma_start(dbg_hf[:], hF[j][:])
                    tconv = workb.tile([128, 3 * 128], f32, tag="tconv")
                    nc.vector.tensor_copy(tconv[:], hT[j][:])
                    nc.sync.dma_start(dbg_ht[:], tconv[:])

            for t in range(n_steps):
                warm = t < n_warm
                if warm:
                    if t in xpre:
                        xtile = xpre.pop(t)
                    else:
                        xtile = xs.tile([128, 4 * B], bf16, tag="xt")
                        for c in range(4):
                            nc.sync.dma_start(
                                xtile[:, c * B : (c + 1) * B],
                                xt[t * 512 + c * 128 : t * 512 + (c + 1) * 128, :],
                            )

                    def gx0_lhsT(c, _x=xtile):
                        return (
                            ones_t[:, 0:64]
                            if c >= 4
                            else _x[:, c * B : c * B + B]
                        )

                    if debug and t == 0:
                        xconv = workb.tile([128, 4 * B], f32, tag="xconv")
                        nc.vector.tensor_copy(xconv[:], xtile[:])
                        nc.sync.dma_start(dbg_xt[:], xconv[:])

                    gru_layer(0, wx0_t, wx0_c, gx0_lhsT)
                    if t == n_warm - 1:
                        # overwrite the shared slot with Weff for the AR phase
                        for c in range(weff_c):
                            nc.sync.dma_start(
                                wshare_t[:, c * G : (c + 1) * G],
                                weff[c * 128 : (c + 1) * 128, :],
                            )
                else:

                    def gxar_lhsT(c):
                        return ones_t[:, 0:64] if c >= 6 else ht_slice(hT[2], c)

                    gru_layer(0, weff_t, weff_c, gxar_lhsT)

                for j in (1, 2):

                    def gx_lhsT(c, _j=j):
                        return ones_t[:, 0:64] if c >= 6 else ht_slice(hT[_j - 1], c)

                    gru_layer(j, wx_t[j], wx[j][1], gx_lhsT)

                # dense readout: pred = p2 @ Wd (+bd), off the critical path
                if t >= n_warm - 1:
                    prd = paux.tile([64, 512], f32, tag="aux")
                    for c in range(wd_c):
                        lhsT = ones_t[:, 0:64] if c >= 6 else ht_slice(hT[2], c)
                        nc.tensor.matmul(
                            prd[0:64, :],
                            lhsT,
                            wd_t[:, c * D : (c + 1) * D],
                            start=c == 0,
                            stop=c == wd_c - 1,
                        )
                    prs = workb.tile([64, 512], f32, tag="pred")
                    nc.vector.tensor_copy(prs[:], prd[:])
                    nc.sync.dma_start(out[:, t - (n_warm - 1), :], prs[:])
    nc.finalize()
    return nc


def kernel(**inputs):
    x = np.asarray(inputs["inputs"], np.float32)
    n_warm, n_ar = T_IN, T_OUT - 1
    x = x[:, :n_warm, :]

    mean = np.asarray(inputs["mean"], np.float32)[0]
    std = np.asarray(inputs["std"], np.float32)[0]
    wd_m = np.asarray(inputs["Wd"], np.float32)
    bd = np.asarray(inputs["bd"], np.float32)
    w1 = np.asarray(inputs["Wx0"], np.float32) / std[:, None]
    weff_m = wd_m @ w1
    beff = (bd - mean) @ w1 + np.asarray(inputs["bi0"], np.float32)

    bias_flags = {}
    wx0_a, bias_flags["bi0"] = _prep_weight(
        np.asarray(inputs["Wx0"], np.float32), np.asarray(inputs["bi0"], np.float32)
    )
    weff_a, has_beff = _prep_weight(weff_m, beff)
    bias_flags["beff"] = has_beff
    wx_a = {}
    wh_a = {}
    for j in range(3):
        if j > 0:
            wx_a[j], bias_flags[f"bi{j}"] = _prep_weight(
                np.asarray(inputs[f"Wx{j}"], np.float32),
                np.asarray(inputs[f"bi{j}"], np.float32),
            )
        wh_a[j], bias_flags[f"br{j}"] = _prep_weight(
            np.asarray(inputs[f"Wh{j}"], np.float32),
            np.asarray(inputs[f"br{j}"], np.float32),
        )
    # dense readout chunks (no column permutation)
    wd_p = wd_m.reshape(6, 128, D)
    bias_flags["bd"] = float(np.abs(bd).max()) > 0.0
    if bias_flags["bd"]:
        bc = np.zeros((1, 128, D), np.float32)
        bc[0, 0, :] = bd
        wd_p = np.concatenate([wd_p, bc], axis=0)
    wd_a = wd_p.astype(BF16)

    # warmup inputs, transposed per step: [T, D, B] -> [T*4*128, B]
    xt_a = np.ascontiguousarray(x.transpose(1, 2, 0)).reshape(n_warm * 512, B)
    xt_a = xt_a.astype(BF16)

    h0f_l = []
    h0t_l = []
    for j in range(3):
        h0 = np.tile(np.asarray(inputs[f"h0_{j}"], np.float32), (B, 1))
        f = _fold(h0)  # [128, 384]
        h0f_l.append(f)
        tchunks = [f[:, c * 128 : (c + 1) * 128].T for c in range(3)]
        h0t_l.append(np.concatenate(tchunks, axis=1))
    h0f_a = np.concatenate(h0f_l, axis=0).astype(np.float32)
    h0t_a = np.concatenate(h0t_l, axis=0).astype(BF16)

    ones_a = np.zeros((128, 128), np.float32)
    ones_a[0, :] = 1.0
    ones_a = ones_a.astype(BF16)

    nc = _build(n_warm, n_ar, bias_flags)
    in_map = {
        "wx0": wx0_a.reshape(-1, G),
        "weff": weff_a.reshape(-1, G),
        "wx1": wx_a[1].reshape(-1, G),
        "wx2": wx_a[2].reshape(-1, G),
        "wh0": wh_a[0].reshape(-1, G),
        "wh1": wh_a[1].reshape(-1, G),
        "wh2": wh_a[2].reshape(-1, G),
        "wd": wd_a.reshape(-1, D),
        "xt": xt_a,
        "h0f": h0f_a,
        "h0t": h0t_a,
        "ones": ones_a,
    }
    res = run_bass_kernel_spmd(
        nc,
        [in_map],
        core_ids=[0],
        trace=os.environ.get("GRU_TRACE", "") == "1",
    )
    kernel._last = res
    kernel._last_nc = nc
    return np.asarray(res.results[0]["out"], np.float32)


if __name__ == "__main__":
    rng = np.random.RandomState(0)
    print("smoke build only")


# revision 37
# speedup vs baseline: 1.0113x; 1.0113x over previous
"""Trainium2 Bass kernel for the 3-layer AR GRU (nn_AR_RNN_GRU).

Strategy
--------
The time recurrence is strictly sequential (127 dependent steps x 3 layers),
and cross-core exchange on this part costs more than it saves (ncfw collective
floor ~5-9us vs ~3us of per-layer compute; remote SBUF-to-SBUF DMA is not
available under this runtime).  So the whole recurrence runs on ONE core with
the full batch of 64, organized to keep the PE and the vector engines busy:

 * "Folded" layout: a [64, 768] activation lives as [128, 384] in SBUF --
   batch on partitions 0-63 for units 0-383 and partitions 64-127 for units
   384-767.  All elementwise gate math then uses the full 128 lanes, and each
   weight matrix streams as two 1152-column halves through the two PE
   column-group pairs concurrently (tile_position via psum base partition).
 * Weights and matmul stationaries live in SBUF as fp16 (fp32 does not fit
   in SBUF; fp16 keeps the 127-step compounded error ~1e-2 where bf16 gave
   ~8e-2); the folded recurrent state and all PSUM accumulation stay fp32.
 * Gate pre-activations: psum_zr accumulates x@Wx + h@Wh for the z,r gates
   (the add is free in PSUM); the candidate keeps xh and hh separate so that
   hc = tanh(xh + r*hh) matches the reset_after GRU cell.
 * The AR feedback (normalize + dense) is folded into one effective matrix:
   gx0 = p2 @ (Wd @ (Wx0/std)) + beff, which removes the dense+normalize from
   the critical path; the actual prediction p2 @ Wd + bd is computed off-path.
 * h is re-transposed each layer via 3 PE-transposes of [128,128] tiles; the
   transposed tiles double as the lhsT (stationary) for the next matmuls and
   for the dense readout.
Biases are applied exactly via an extra "ones" K-chunk whose rhs row 0 holds
the bias vector -- emitted only when the bias is nonzero (in this problem all
bi/br/bd are zero; beff is nonzero and always emitted).
"""

import os
import sys

import numpy as np

try:
    import concourse.bass as bass  # noqa: F401
except ImportError:  # grading env fallback
    sys.path.insert(0, "/opt/trn_rl_repo")

import ml_dtypes

import concourse.bass as bass
import concourse.mybir as mybir
import concourse.tile as tile
from concourse import bacc
from concourse.bass_utils import run_bass_kernel_spmd
from concourse.masks import make_identity

BF16 = np.float16

B = 64  # batch
D = 512  # data dim
U = 768  # GRU units
G = 3 * U  # gate columns
HALF = U // 2  # 384

T_IN = int(os.environ.get("GRU_TIN", "64"))
T_OUT = int(os.environ.get("GRU_TOUT", "64"))

# column permutation: [z_lo r_lo h_lo | z_hi r_hi h_hi], each block 384 wide
_PERM = np.concatenate(
    [
        np.arange(0, HALF),
        np.arange(U, U + HALF),
        np.arange(2 * U, 2 * U + HALF),
        np.arange(HALF, U),
        np.arange(U + HALF, 2 * U),
        np.arange(2 * U + HALF, G),
    ]
)


def _prep_weight(w, bias):
    """[K, 2304] fp32 (+bias [2304]) -> ([n_chunks, 128, 2304] bf16, has_bias)."""
    k = w.shape[0]
    assert k % 128 == 0
    wp = np.ascontiguousarray(w[:, _PERM]).reshape(k // 128, 128, G)
    has_bias = bias is not None and float(np.abs(bias).max()) > 0.0
    if has_bias:
        bc = np.zeros((1, 128, G), np.float32)
        bc[0, 0, :] = bias[_PERM]
        wp = np.concatenate([wp, bc], axis=0)
    return wp.astype(BF16), has_bias


def _fold(a):
    """[64, 768] -> folded [128, 384]."""
    return np.concatenate([a[:, :HALF], a[:, HALF:]], axis=0)


def _build(n_warm, n_ar, bias_flags):
    """Build the Bass program. bias_flags: dict name->bool for extra chunks."""
    nc = bacc.Bacc(num_devices=1, name="gru_ar")
    f32, bf16 = mybir.dt.float32, mybir.dt.float16
    n_steps = n_warm + n_ar

    # ---- DRAM I/O ----
    def wchunks(name, kc):
        return nc.dram_tensor(name, [kc * 128, G], bf16, kind="ExternalInput"), kc

    wx0, wx0_c = wchunks("wx0", 4 + bias_flags["bi0"])
    weff, weff_c = wchunks("weff", 6 + bias_flags["beff"])
    wx = [None, *(wchunks(f"wx{j}", 6 + bias_flags[f"bi{j}"]) for j in (1, 2))]
    wh = [wchunks(f"wh{j}", 6 + bias_flags[f"br{j}"]) for j in (0, 1, 2)]
    wd_c = 6 + bias_flags["bd"]
    wd = nc.dram_tensor("wd", [wd_c * 128, D], bf16, kind="ExternalInput")
    xt = nc.dram_tensor("xt", [n_warm * 4 * 128, B], bf16, kind="ExternalInput")
    h0f = nc.dram_tensor("h0f", [3 * 128, HALF], f32, kind="ExternalInput")
    h0t = nc.dram_tensor("h0t", [3 * 128, 3 * 128], bf16, kind="ExternalInput")
    ones = nc.dram_tensor("ones", [128, 128], bf16, kind="ExternalInput")
    out = nc.dram_tensor("out", [B, n_ar + 1, D], f32, kind="ExternalOutput")
    debug = os.environ.get("GRU_DEBUG", "") == "1"
    if debug:
        dbg_zr = nc.dram_tensor("dbg_zr", [128, 2 * HALF], f32, kind="ExternalOutput")
        dbg_hf = nc.dram_tensor("dbg_hf", [128, HALF], f32, kind="ExternalOutput")
        dbg_xt = nc.dram_tensor("dbg_xt", [128, 4 * B], f32, kind="ExternalOutput")
        dbg_ht = nc.dram_tensor("dbg_ht", [128, 3 * 128], f32, kind="ExternalOutput")

    with tile.TileContext(nc) as tc:
        with (
            tc.tile_pool(name="wpool", bufs=1) as wpool,
            tc.tile_pool(name="state", bufs=1) as spool,
            tc.tile_pool(name="work", bufs=2) as work,
            tc.tile_pool(name="workb", bufs=2) as workb,
            tc.tile_pool(name="xs", bufs=3) as xs,
            tc.tile_pool(name="pzr", bufs=2, space="PSUM") as pzr,
            tc.tile_pool(name="ph", bufs=2, space="PSUM") as ph,
            tc.tile_pool(name="paux", bufs=2, space="PSUM") as paux,
        ):
            # ---- load weights / constants ----
            def load_w(dram, kc, ncols):
                t = wpool.tile([128, kc * ncols], bf16, tag=dram.name)
                for c in range(kc):
                    nc.sync.dma_start(
                        t[:, c * ncols : (c + 1) * ncols],
                        dram[c * 128 : (c + 1) * 128, :],
                    )
                return t

            # Wx0 (warmup) and Weff (AR) share one SBUF slot; Weff is DMA'd
            # over Wx0 after the last warmup gx0 matmul (Tile orders the WAR).
            # small step-0 state/constants first, then weights in the order
            # the first step consumes them (wx0, wh0, wx1, wh1, ...), so the
            # PE can start while the rest of the ~21 MB is still in flight
            ones_t = wpool.tile([128, 128], bf16, tag="ones")
            nc.sync.dma_start(ones_t[:], ones[:])
            ident = wpool.tile([128, 128], f32, tag="ident")
            make_identity(nc, ident[:])
            ident16 = wpool.tile([128, 128], bf16, tag="ident16")
            nc.vector.tensor_copy(ident16[:], ident[:])
            hF = []
            hT = []
            for j in range(3):
                f = spool.tile([128, HALF], f32, tag=f"hF{j}")
                nc.sync.dma_start(f[:], h0f[j * 128 : (j + 1) * 128, :])
                hF.append(f)
                t = spool.tile([128, 3 * 128], bf16, tag=f"hT{j}")
                nc.sync.dma_start(t[:], h0t[j * 128 : (j + 1) * 128, :])
                hT.append(t)

            nshare = max(wx0_c, weff_c)
            wshare_t = wpool.tile([128, nshare * G], bf16, tag="wx0weff")
            for c in range(wx0_c):
                nc.sync.dma_start(
                    wshare_t[:, c * G : (c + 1) * G], wx0[c * 128 : (c + 1) * 128, :]
                )
            wx0_t = wshare_t
            weff_t = wshare_t
            wh_t = [None, None, None]
            wx_t = [None, None, None]
            wh_t[0] = load_w(wh[0][0], wh[0][1], G)
            # prefetch the first two warmup x tiles ahead of the bulk weights
            xpre = {}
            for tt in range(min(2, n_warm)):
                xtile = xs.tile([128, 4 * B], bf16, tag="xt")
                for c in range(4):
                    nc.sync.dma_start(
                        xtile[:, c * B : (c + 1) * B],
                        xt[tt * 512 + c * 128 : tt * 512 + (c + 1) * 128, :],
                    )
                xpre[tt] = xtile
            wx_t[1] = load_w(wx[1][0], wx[1][1], G)
            wh_t[1] = load_w(wh[1][0], wh[1][1], G)
            wx_t[2] = load_w(wx[2][0], wx[2][1], G)
            wh_t[2] = load_w(wh[2][0], wh[2][1], G)
            wd_t = load_w(wd, wd_c, D)

            def ht_slice(t, c):
                # K-chunk c (0..5) of the transposed folded state tile set
                if c < 3:
                    return t[:, c * 128 : c * 128 + 64]
                return t[:, (c - 3) * 128 + 64 : (c - 3) * 128 + 128]

            # ---- one recurrent step ----
            def stream(psum_zr, psum_h, w_t, kc, lhsT_fn, first_zr, last_zr):
                """Emit the matmuls of one weight stream (both folded halves).
                6-chunk streams go in order 0,3,1,4,2,5 so that each transposed
                state tile T_c unlocks its two K-chunks (c, c+3) as it lands."""
                order = [0, 3, 1, 4, 2, 5] + list(range(6, kc)) if kc >= 6 else list(range(kc))
                for ci, c in enumerate(order):
                    lhsT = lhsT_fn(c)
                    for h_ix in range(2):
                        base = 64 * h_ix
                        off = h_ix * (G // 2)
                        first = first_zr and ci == 0
                        last = last_zr and ci == kc - 1
                        nc.tensor.matmul(
                            psum_zr[base : base + 64, 0:512],
                            lhsT,
                            w_t[:, c * G + off : c * G + off + 512],
                            start=first,
                            stop=last,
                        )
                        nc.tensor.matmul(
                            psum_zr[base : base + 64, 512:768],
                            lhsT,
                            w_t[:, c * G + off + 512 : c * G + off + 768],
                            start=first,
                            stop=last,
                        )
                        nc.tensor.matmul(
                            psum_h[base : base + 64, 0:HALF],
                            lhsT,
                            w_t[:, c * G + off + 768 : c * G + off + 1152],
                            start=ci == 0,
                            stop=ci == kc - 1,
                        )

            def gru_layer(j, gx_w, gx_kc, gx_lhsT_fn):
                psum_zr = pzr.tile([128, 2 * HALF], f32, tag="zr")
                psum_xh = ph.tile([128, HALF], f32, tag="xh")
                psum_hh = paux.tile([128, HALF], f32, tag="aux")
                # recurrent stream first (inputs available earliest)
                wh_kc = wh[j][1]

                def gh_lhsT(c):
                    return ones_t[:, 0:64] if c >= 6 else ht_slice(hT[j], c)

                stream(psum_zr, psum_hh, wh_t[j], wh_kc, gh_lhsT, True, False)
                stream(psum_zr, psum_xh, gx_w, gx_kc, gx_lhsT_fn, False, True)

                # gates (folded [128, *])
                zr = work.tile([128, 2 * HALF], f32, tag="zr_s")
                # r first: it gates the critical path (r*hh); z can lag
                nc.scalar.activation(
                    zr[:, HALF : 2 * HALF],
                    psum_zr[:, HALF : 2 * HALF],
                    mybir.ActivationFunctionType.Sigmoid,
                )
                t1 = workb.tile([128, HALF], f32, tag="t1")
                nc.vector.tensor_mul(t1[:], zr[:, HALF : 2 * HALF], psum_hh[:])
                nc.scalar.activation(
                    zr[:, 0:HALF],
                    psum_zr[:, 0:HALF],
                    mybir.ActivationFunctionType.Sigmoid,
                )
                if debug and j == 0:
                    nc.sync.dma_start(dbg_zr[:], zr[:])
                nc.vector.tensor_add(t1[:], t1[:], psum_xh[:])
                hc = workb.tile([128, HALF], f32, tag="hc")
                nc.scalar.activation(hc[:], t1[:], mybir.ActivationFunctionType.Tanh)
                # h_new = hc + z*(h_prev - hc)
                d = workb.tile([128, HALF], f32, tag="d")
                nc.vector.tensor_sub(d[:], hF[j][:], hc[:])
                nc.vector.tensor_mul(d[:], zr[:, 0:HALF], d[:])
                nc.vector.tensor_add(hF[j][:], d[:], hc[:])
                # duplicate fp16 state write: lets the transposes run at
                # 1 cyc/row (fp16) with bit-identical hT (same rounding point)
                h16 = workb.tile([128, HALF], bf16, tag="h16")
                nc.vector.tensor_add(h16[:], d[:], hc[:])
                # re-transpose the folded state for the next matmuls
                for c in range(3):
                    ptr = ph.tile([128, 128], bf16, tag="xh")
                    nc.tensor.transpose(
                        ptr[:], h16[:, c * 128 : (c + 1) * 128], ident16[:]
                    )
                    # alternate engines so psum->sbuf copies run in parallel
                    if c == 1:
                        nc.vector.tensor_copy(hT[j][:, c * 128 : (c + 1) * 128], ptr[:])
                    else:
                        nc.scalar.copy(hT[j][:, c * 128 : (c + 1) * 128], ptr[:])
                if debug and j == 0:
                    nc.sync.dma_start(dbg_hf[:], hF[j][:])
                    tconv = workb.tile([128, 3 * 128], f32, tag="tconv")
                    nc.vector.tensor_copy(tconv[:], hT[j][:])
                    nc.sync.dma_start(dbg_ht[:], tconv[:])

            for t in range(n_steps):
                warm = t < n_warm
                if warm:
                    if t in xpre:
                        xtile = xpre.pop(t)
                    else:
                        xtile = xs.tile([128, 4 * B], bf16, tag="xt")
                        for c in range(4):
                            nc.sync.dma_start(
                                xtile[:, c * B : (c + 1) * B],
                                xt[t * 512 + c * 128 : t * 512 + (c + 1) * 128, :],
                            )

                    def gx0_lhsT(c, _x=xtile):
                        return (
                            ones_t[:, 0:64]
                            if c >= 4
                            else _x[:, c * B : c * B + B]
                        )

                    if debug and t == 0:
                        xconv = workb.tile([128, 4 * B], f32, tag="xconv")
                        nc.vector.tensor_copy(xconv[:], xtile[:])
                        nc.sync.dma_start(dbg_xt[:], xconv[:])

                    gru_layer(0, wx0_t, wx0_c, gx0_lhsT)
                    if t == n_warm - 1:
                        # overwrite the shared slot with Weff for the AR phase
                        for c in range(weff_c):
                            nc.sync.dma_start(
                                wshare_t[:, c * G : (c + 1) * G],
                                weff[c * 128 : (c + 1) * 128, :],
                            )
                else:

                    def gxar_lhsT(c):
                        return ones_t[:, 0:64] if c >= 6 else ht_slice(hT[2], c)

                    gru_layer(0, weff_t, weff_c, gxar_lhsT)

                for j in (1, 2):

                    def gx_lhsT(c, _j=j):
                        return ones_t[:, 0:64] if c >= 6 else ht_slice(hT[_j - 1], c)

                    gru_layer(j, wx_t[j], wx[j][1], gx_lhsT)

                # dense readout: pred = p2 @ Wd (+bd), off the critical path
                if t >= n_warm - 1:
                    prd = paux.tile([64, 512], f32, tag="aux")
                    for c in range(wd_c):
                        lhsT = ones_t[:, 0:64] if c >= 6 else ht_slice(hT[2], c)
                        nc.tensor.matmul(
                            prd[0:64, :],
                            lhsT,
                            wd_t[:, c * D : (c + 1) * D],
                            start=c == 0,
                            stop=c == wd_c - 1,
                        )
                    prs = workb.tile([64, 512], f32, tag="pred")
                    nc.vector.tensor_copy(prs[:], prd[:])
                    nc.sync.dma_start(out[:, t - (n_warm - 1), :], prs[:])
    nc.finalize()
    return nc


def kernel(**inputs):
    x = np.asarray(inputs["inputs"], np.float32)
    n_warm, n_ar = T_IN, T_OUT - 1
    x = x[:, :n_warm, :]

    mean = np.asarray(inputs["mean"], np.float32)[0]
    std = np.asarray(inputs["std"], np.float32)[0]
    wd_m = np.asarray(inputs["Wd"], np.float32)
    bd = np.asarray(inputs["bd"], np.float32)
    w1 = np.asarray(inputs["Wx0"], np.float32) / std[:, None]
    weff_m = wd_m @ w1
    beff = (bd - mean) @ w1 + np.asarray(inputs["bi0"], np.float32)

    bias_flags = {}
    wx0_a, bias_flags["bi0"] = _prep_weight(
        np.asarray(inputs["Wx0"], np.float32), np.asarray(inputs["bi0"], np.float32)
    )
    weff_a, has_beff = _prep_weight(weff_m, beff)
    bias_flags["beff"] = has_beff
    wx_a = {}
    wh_a = {}
    for j in range(3):
        if j > 0:
            wx_a[j], bias_flags[f"bi{j}"] = _prep_weight(
                np.asarray(inputs[f"Wx{j}"], np.float32),
                np.asarray(inputs[f"bi{j}"], np.float32),
            )
        wh_a[j], bias_flags[f"br{j}"] = _prep_weight(
            np.asarray(inputs[f"Wh{j}"], np.float32),
            np.asarray(inputs[f"br{j}"], np.float32),
        )
    # dense readout chunks (no column permutation)
    wd_p = wd_m.reshape(6, 128, D)
    bias_flags["bd"] = float(np.abs(bd).max()) > 0.0
    if bias_flags["bd"]:
        bc = np.zeros((1, 128, D), np.float32)
        bc[0, 0, :] = bd
        wd_p = np.concatenate([wd_p, bc], axis=0)
    wd_a = wd_p.astype(BF16)

    # warmup inputs, transposed per step: [T, D, B] -> [T*4*128, B]
    xt_a = np.ascontiguousarray(x.transpose(1, 2, 0)).reshape(n_warm * 512, B)
    xt_a = xt_a.astype(BF16)

    h0f_l = []
    h0t_l = []
    for j in range(3):
        h0 = np.tile(np.asarray(inputs[f"h0_{j}"], np.float32), (B, 1))
        f = _fold(h0)  # [128, 384]
        h0f_l.append(f)
        tchunks = [f[:, c * 128 : (c + 1) * 128].T for c in range(3)]
        h0t_l.append(np.concatenate(tchunks, axis=1))
    h0f_a = np.concatenate(h0f_l, axis=0).astype(np.float32)
    h0t_a = np.concatenate(h0t_l, axis=0).astype(BF16)

    ones_a = np.zeros((128, 128), np.float32)
    ones_a[0, :] = 1.0
    ones_a = ones_a.astype(BF16)

    nc = _build(n_warm, n_ar, bias_flags)
    in_map = {
        "wx0": wx0_a.reshape(-1, G),
        "weff": weff_a.reshape(-1, G),
        "wx1": wx_a[1].reshape(-1, G),
        "wx2": wx_a[2].reshape(-1, G),
        "wh0": wh_a[0].reshape(-1, G),
        "wh1": wh_a[1].reshape(-1, G),
        "wh2": wh_a[2].reshape(-1, G),
        "wd": wd_a.reshape(-1, D),
        "xt": xt_a,
        "h0f": h0f_a,
        "h0t": h0t_a,
        "ones": ones_a,
    }
    res = run_bass_kernel_spmd(
        nc,
        [in_map],
        core_ids=[0],
        trace=os.environ.get("GRU_TRACE", "") == "1",
    )
    kernel._last = res
    kernel._last_nc = nc
    return np.asarray(res.results[0]["out"], np.float32)


if __name__ == "__main__":
    rng = np.random.RandomState(0)
    print("smoke build only")
